# revision 11
# baseline (speedup 1.0000x reference)
"""Trainium2 Bass kernel for nn_ApproachNet_view_fps_objectness.

Strategy (8 NeuronCores, data-parallel over batch, 2 cores per batch element):

  NEFF-A: graspable head (conv1x1 256->256, BN+ReLU folded, conv1x1 256->3)
          core c handles batch c//2, N-half c%2 (10000 points).
  host:   objectness/graspness masks from device scores (exact comparisons),
          compacted masked-point index lists (only ~170 points pass the mask).
  NEFF-B: masked FPS over the compacted point set (sequential argmax loop with
          T = max(M_b) iterations, exact f32 semantics matching the reference),
          then gathers + view head convs + view argmax + rotation matrices.
          Both cores of a batch pair run FPS redundantly; the view head is
          split 512 points per core.

All selection-critical arithmetic (mask compares, FPS distance updates and
argmaxes, view argmax) is bit-exact w.r.t. the reference ops ordering.
"""
import os
import sys
import math

import numpy as np

if "/opt/trn_rl_repo" not in sys.path:
    sys.path.insert(0, "/opt/trn_rl_repo")

import concourse.bass as bass
import concourse.bacc as bacc
import concourse.mybir as mybir
import concourse.tile as tile
from concourse.bass_utils import run_bass_kernel_spmd

F32 = mybir.dt.float32
U32 = mybir.dt.uint32
I32 = mybir.dt.int32
AL = mybir.AluOpType
DVE = [mybir.EngineType.DVE]
PE = [mybir.EngineType.PE]
SP = [mybir.EngineType.SP]

B, N, CH, V, NS = 4, 20000, 256, 300, 1024
NH = N // 2              # per-core half of the point cloud
NT = NH // 512           # 512-point conv tiles (NH divisible: 10000/512 no...)
# 10000 = 19*512 + 288 -> use tiles of 500: 20 tiles
TILE_A = 500
NTA = NH // TILE_A       # 20
P, FC = 64, 8            # FPS compacted layout: 64 partitions x 8 slots
CCAP = P * FC            # 512 compacted-point capacity
EPS = np.float32(1e-5)

LAST_TIMES = {}

_prog_cache = {}


def _fold_bn(w, b, g, be, m, v):
    s = (g / np.sqrt(v + EPS)).astype(np.float32)
    return (w * s[:, None]).astype(np.float32), (s * (b - m) + be).astype(np.float32)


def _chunk_lhsT(wT, kchunks, mtot):
    # wT [K, M] -> [128, kchunks*M] with chunk kc at cols [kc*M:(kc+1)*M]
    K, M = wT.shape
    kpad = kchunks * 128
    out = np.zeros((kpad, M), np.float32)
    out[:K] = wT
    return np.ascontiguousarray(
        out.reshape(kchunks, 128, M).transpose(1, 0, 2).reshape(128, kchunks * M)
    )


def _chunk_bias(b, mchunks):
    # b [M] -> [128, mchunks], chunk mc in column mc (zero padded)
    out = np.zeros((mchunks * 128,), np.float32)
    out[: b.shape[0]] = b
    return np.ascontiguousarray(out.reshape(mchunks, 128).T)


def _templates():
    phi = (np.sqrt(5.0) - 1.0) / 2.0
    i = np.arange(V)
    zi = (2 * i + 1) / V - 1
    ri = np.sqrt(np.maximum(1 - zi**2, 0))
    xi = ri * np.cos(2 * np.pi * i * phi)
    yi = ri * np.sin(2 * np.pi * i * phi)
    return np.stack([xi, yi, zi], axis=-1).astype(np.float32)


def build_neff_a():
    nc = bacc.Bacc("TRN2", target_bir_lowering=False, debug=False, num_devices=8)
    fsh = nc.dram_tensor("fsh", [CH, NH], F32, kind="ExternalInput").ap()
    w1sb_d = nc.dram_tensor("w1c", [128, 2 * 256], F32, kind="ExternalInput").ap()
    b1sb_d = nc.dram_tensor("b1c", [128, 2], F32, kind="ExternalInput").ap()
    w2sb_d = nc.dram_tensor("w2c", [128, 2 * 3], F32, kind="ExternalInput").ap()
    b2sb_d = nc.dram_tensor("b2c", [3, 1], F32, kind="ExternalInput").ap()
    g_o = nc.dram_tensor("g_o", [3, NH], F32, kind="ExternalOutput").ap()

    with tile.TileContext(nc) as tc:
        with (
            tc.tile_pool(name="wp", bufs=1) as wp,
            tc.tile_pool(name="io", bufs=3) as io,
            tc.tile_pool(name="hp", bufs=3) as hp,
            tc.tile_pool(name="pp", bufs=4, space="PSUM") as pp,
            tc.tile_pool(name="pp2", bufs=2, space="PSUM") as pp2,
        ):
            w1sb = wp.tile([128, 2 * 256], F32, tag="w1sb")
            b1sb = wp.tile([128, 2], F32, tag="b1sb")
            w2sb = wp.tile([128, 2 * 3], F32, tag="w2sb")
            b2sb = wp.tile([3, 1], F32, tag="b2sb")
            nc.sync.dma_start(w1sb[:], w1sb_d)
            nc.sync.dma_start(b1sb[:], b1sb_d)
            nc.sync.dma_start(w2sb[:], w2sb_d)
            nc.sync.dma_start(b2sb[:], b2sb_d)

            for nt in range(NTA):
                sl = slice(nt * TILE_A, (nt + 1) * TILE_A)
                ft = io.tile([128, 2, TILE_A], F32, tag="ft")
                nc.sync.dma_start(
                    ft[:], fsh[:, sl].rearrange("(c p) n -> p c n", p=128)
                )
                h1 = hp.tile([128, 2, TILE_A], F32, tag="h1")
                for mc in range(2):
                    ps = pp.tile([128, TILE_A], F32, tag="ps")
                    for kc in range(2):
                        nc.tensor.matmul(
                            out=ps[:],
                            lhsT=w1sb[:, kc * 256 + mc * 128 : kc * 256 + (mc + 1) * 128],
                            rhs=ft[:, kc, :],
                            start=(kc == 0),
                            stop=(kc == 1),
                        )
                    # relu(x + bias) in one exact DVE op
                    nc.vector.tensor_scalar(
                        h1[:, mc, :], ps[:], b1sb[:, mc : mc + 1], 0.0, AL.add, AL.max
                    )
                ps2 = pp2.tile([3, TILE_A], F32, tag="ps2")
                for kc in range(2):
                    nc.tensor.matmul(
                        out=ps2[:],
                        lhsT=w2sb[:, kc * 3 : (kc + 1) * 3],
                        rhs=h1[:, kc, :],
                        start=(kc == 0),
                        stop=(kc == 1),
                    )
                gt = io.tile([3, TILE_A], F32, tag="gt")
                nc.vector.tensor_scalar(gt[:], ps2[:], b2sb[:, 0:1], None, AL.add)
                nc.sync.dma_start(g_o[:, sl], gt[:])
    nc.finalize()
    return nc


def build_neff_b(T):
    nc = bacc.Bacc("TRN2", target_bir_lowering=False, debug=False, num_devices=8)
    xyz_d = nc.dram_tensor("xyz", [N, 3], F32, kind="ExternalInput").ap()
    fT_d = nc.dram_tensor("fT", [N, CH], F32, kind="ExternalInput").ap()
    g2_d = nc.dram_tensor("g2f", [N, 1], F32, kind="ExternalInput").ap()
    tmpl_d = nc.dram_tensor("tmpl", [V, 3], F32, kind="ExternalInput").ap()
    midxP_d = nc.dram_tensor("midxP", [P, FC], U32, kind="ExternalInput").ap()
    midxR_d = nc.dram_tensor("midxR", [1, CCAP], U32, kind="ExternalInput").ap()
    midxC_d = nc.dram_tensor("midxC", [CCAP, 1], U32, kind="ExternalInput").ap()
    dist0_d = nc.dram_tensor("dist0", [P, FC], F32, kind="ExternalInput").ap()
    hoff_d = nc.dram_tensor("hoff", [1, 1], U32, kind="ExternalInput").ap()
    ident_d = nc.dram_tensor("ident", [128, 128], F32, kind="ExternalInput").ap()
    iota8_d = nc.dram_tensor("iota8", [P, 1], F32, kind="ExternalInput").ap()
    w1v_d = nc.dram_tensor("w1v", [128, 2 * 256], F32, kind="ExternalInput").ap()
    b1v_d = nc.dram_tensor("b1v", [128, 2], F32, kind="ExternalInput").ap()
    w2v_d = nc.dram_tensor("w2v", [128, 2 * 300], F32, kind="ExternalInput").ap()
    b2v_d = nc.dram_tensor("b2v", [128, 3], F32, kind="ExternalInput").ap()
    w3v_d = nc.dram_tensor("w3v", [128, 3 * 300], F32, kind="ExternalInput").ap()
    b3v_d = nc.dram_tensor("b3v", [128, 3], F32, kind="ExternalInput").ap()

    inds_o = nc.dram_tensor("inds_o", [1, NS], U32, kind="ExternalOutput").ap()
    feat_o = nc.dram_tensor("feat_o", [CH, 512], F32, kind="ExternalOutput").ap()
    vs_o = nc.dram_tensor("vs_o", [512, V], F32, kind="ExternalOutput").ap()
    tvi_o = nc.dram_tensor("tvi_o", [128, 4], U32, kind="ExternalOutput").ap()
    tvs_o = nc.dram_tensor("tvs_o", [128, 4], F32, kind="ExternalOutput").ap()
    vpx_o = nc.dram_tensor("vpx_o", [128, 4, 3], F32, kind="ExternalOutput").ap()
    vpr_o = nc.dram_tensor("vpr_o", [128, 4, 9], F32, kind="ExternalOutput").ap()
    fp2_o = nc.dram_tensor("fp2_o", [128, 4], F32, kind="ExternalOutput").ap()
    gxyz_o = nc.dram_tensor("gxyz_o", [128, 4, 3], F32, kind="ExternalOutput").ap()

    scratch = nc.dram_tensor("scratch", [1, 3 * CCAP], F32)

    with tile.TileContext(nc) as tc:
        with (
            tc.tile_pool(name="sb", bufs=1) as sb,
            tc.tile_pool(name="ps", bufs=2, space="PSUM") as psp,
            tc.tile_pool(name="mm", bufs=3, space="PSUM") as mmp,
        ):
            # ---------- persistent tiles ----------
            pts = sb.tile([P, FC, 3], F32, tag="pts")
            xyzitl = sb.tile([1, 3 * CCAP], F32, tag="xyzitl")
            dist = sb.tile([P, FC], F32, tag="dist")
            midxsb = sb.tile([1, CCAP], U32, tag="midxsb")
            midxPsb = sb.tile([P, FC], U32, tag="midxPsb")
            tr = sb.tile([P, 2], F32, tag="tr")
            zeros8 = sb.tile([P, FC], F32, tag="zeros8")
            mrep = sb.tile([P, FC], F32, tag="mrep")
            i8t = sb.tile([P, 8], U32, tag="i8t")
            m1row = sb.tile([1, P], F32, tag="m1row")
            cmb = sb.tile([1, P], U32, tag="cmb")
            gm8t = sb.tile([1, 8], F32, tag="gm8t")
            gp8t = sb.tile([1, 8], U32, tag="gp8t")
            indsrow = sb.tile([1, NS], U32, tag="indsrow")
            ones64 = sb.tile([1, P], F32, tag="ones64")
            idsb = sb.tile([128, 128], F32, tag="idsb")
            iotasb = sb.tile([P, 1], F32, tag="iotasb")
            diff = sb.tile([P, FC, 3], F32, tag="diff")
            sqd = sb.tile([P, FC, 3], F32, tag="sqd")
            dsq = sb.tile([P, FC], F32, tag="dsq")
            hoffsb = sb.tile([1, 1], U32, tag="hoffsb")

            nc.sync.dma_start(midxsb[:], midxR_d)
            nc.sync.dma_start(midxPsb[:], midxP_d)
            nc.sync.dma_start(dist[:], dist0_d)
            nc.sync.dma_start(idsb[:], ident_d)
            nc.sync.dma_start(iotasb[:], iota8_d)
            nc.sync.dma_start(hoffsb[:], hoff_d)
            nc.vector.memset(zeros8[:], 0.0)
            nc.vector.memset(ones64[:], 1.0)

            # compacted coords: one gathered row per partition per slot-column
            for f in range(FC):
                nc.gpsimd.indirect_dma_start(
                    out=pts[:, f, :],
                    out_offset=None,
                    in_=xyz_d,
                    in_offset=bass.IndirectOffsetOnAxis(
                        ap=midxPsb[:, f : f + 1], axis=0
                    ),
                )
            nc.sync.dma_start(
                scratch.ap().rearrange("o (f c) -> o f c", c=3), pts[:]
            )
            nc.sync.dma_start(xyzitl[:], scratch.ap())

            # inds prefilled with midx[0] (covers tail repeats exactly)
            nc.vector.tensor_copy(indsrow[:], midxsb[0:1, 0:1].to_broadcast((1, NS)))

            # ---------- FPS loop ----------
            cb = psp.tile([P, 3], F32, tag="cb")
            nc.tensor.matmul(
                out=cb[:], lhsT=ones64[:], rhs=xyzitl[0:1, 0:3], start=True, stop=True
            )
            for k in range(1, T):
                nc.vector.tensor_tensor(
                    out=diff[:],
                    in0=pts[:],
                    in1=cb[:].rearrange("p (o c) -> p o c", o=1).to_broadcast((P, FC, 3)),
                    op=AL.subtract,
                )
                nc.vector.tensor_tensor(out=sqd[:], in0=diff[:], in1=diff[:], op=AL.mult)
                nc.vector.tensor_reduce(
                    out=dsq[:], in_=sqd[:], axis=mybir.AxisListType.X, op=AL.add
                )
                nc.vector.tensor_tensor(out=dist[:], in0=dist[:], in1=dsq[:], op=AL.min)
                nc.vector.tensor_reduce(
                    out=tr[:, 0:1], in_=dist[:], axis=mybir.AxisListType.X, op=AL.max
                )
                nc.vector.tensor_scalar(mrep[:], zeros8[:], tr[:, 0:1], None, AL.add)
                nc.vector.max_index(i8t[:], mrep[:], dist[:])
                # combo row: 8*p + f  (prepared for single dynamic lookup)
                nc.vector.tensor_scalar(
                    tr[:, 1:2], i8t[:, 0:1], iotasb[:, 0:1], None, AL.add
                )
                trTm = psp.tile([1, P], F32, tag="trT")
                trTc = psp.tile([1, P], F32, tag="trT")
                nc.tensor.transpose(out=trTm[:], in_=tr[:, 0:1], identity=idsb[0:P, 0:P])
                nc.tensor.transpose(out=trTc[:], in_=tr[:, 1:2], identity=idsb[0:P, 0:P])
                nc.vector.tensor_copy(m1row[:], trTm[:])
                nc.vector.tensor_copy(cmb[:], trTc[:])
                nc.vector.max(gm8t[:], m1row[:])
                nc.vector.max_index(gp8t[:], gm8t[:], m1row[:])

                rp = nc.values_load(
                    gp8t[0:1, 0:1].bitcast(I32), engines=DVE,
                    min_val=0, max_val=P - 1, skip_runtime_bounds_check=True,
                )
                rcur = nc.values_load(
                    cmb[0:1, bass.ds(rp, 1)].bitcast(I32), engines=DVE,
                    min_val=0, max_val=CCAP - 1, skip_runtime_bounds_check=True,
                )
                ro = nc.values_load(
                    midxsb[0:1, bass.ds(rcur, 1)].bitcast(I32), engines=DVE,
                    skip_runtime_bounds_check=True,
                )
                nc.vector.store(indsrow[0:1, k : k + 1].bitcast(I32), ro)

                rpP = nc.values_load(
                    gp8t[0:1, 0:1].bitcast(I32), engines=PE,
                    min_val=0, max_val=P - 1, skip_runtime_bounds_check=True,
                )
                rcP = nc.values_load(
                    cmb[0:1, bass.ds(rpP, 1)].bitcast(I32), engines=PE,
                    min_val=0, max_val=CCAP - 1, skip_runtime_bounds_check=True,
                )
                cb = psp.tile([P, 3], F32, tag="cb")
                nc.tensor.matmul(
                    out=cb[:],
                    lhsT=ones64[:],
                    rhs=xyzitl[0:1, bass.ds(rcP * 3, 3)],
                    start=True,
                    stop=True,
                )

            nc.sync.dma_start(inds_o, indsrow[:])

            # ---------- view head (my 512-point half) ----------
            hv = nc.values_load(
                hoffsb[0:1, 0:1].bitcast(I32), engines=SP,
                min_val=0, max_val=512, skip_runtime_bounds_check=True,
            )
            minds4 = sb.tile([128, 4], U32, tag="minds4")
            nc.sync.dma_start(minds4[:], indsrow[0:1, bass.ds(hv, 512)])

            fzT = sb.tile([128, 4, CH], F32, tag="fzT")
            gx = sb.tile([128, 4, 3], F32, tag="gx")
            fp2t = sb.tile([128, 4], F32, tag="fp2t")
            for f in range(4):
                off = bass.IndirectOffsetOnAxis(ap=minds4[:, f : f + 1], axis=0)
                nc.gpsimd.indirect_dma_start(
                    out=fzT[:, f, :], out_offset=None, in_=fT_d, in_offset=off
                )
                nc.gpsimd.indirect_dma_start(
                    out=gx[:, f, :], out_offset=None, in_=xyz_d, in_offset=off
                )
                nc.gpsimd.indirect_dma_start(
                    out=fp2t[:, f : f + 1], out_offset=None, in_=g2_d, in_offset=off
                )
            nc.sync.dma_start(gxyz_o, gx[:])
            nc.sync.dma_start(fp2_o, fp2t[:])

            # weights
            w1v = sb.tile([128, 2 * 256], F32, tag="w1v")
            b1v = sb.tile([128, 2], F32, tag="b1v")
            w2v = sb.tile([128, 2 * 300], F32, tag="w2v")
            b2v = sb.tile([128, 3], F32, tag="b2v")
            w3v = sb.tile([128, 3 * 300], F32, tag="w3v")
            b3v = sb.tile([128, 3], F32, tag="b3v")
            for t_, d_ in ((w1v, w1v_d), (b1v, b1v_d), (w2v, w2v_d),
                           (b2v, b2v_d), (w3v, w3v_d), (b3v, b3v_d)):
                nc.sync.dma_start(t_[:], d_)

            # transpose gathered features [pt, ch] -> [ch, pt]
            fz = sb.tile([128, 2, 512], F32, tag="fz")
            for f in range(4):
                for cc in range(2):
                    tp = mmp.tile([128, 128], F32, tag="psv")
                    nc.tensor.transpose(
                        out=tp[:],
                        in_=fzT[:, f, cc * 128 : (cc + 1) * 128],
                        identity=idsb[:],
                    )
                    nc.vector.tensor_copy(fz[:, cc, f * 128 : (f + 1) * 128], tp[:])

            # conv1: 256 -> 256, relu(bn) folded
            v1 = sb.tile([128, 2, 512], F32, tag="v1")
            for mc in range(2):
                ps = mmp.tile([128, 512], F32, tag="psv")
                for kc in range(2):
                    nc.tensor.matmul(
                        out=ps[:],
                        lhsT=w1v[:, kc * 256 + mc * 128 : kc * 256 + (mc + 1) * 128],
                        rhs=fz[:, kc, :],
                        start=(kc == 0),
                        stop=(kc == 1),
                    )
                nc.vector.tensor_scalar(
                    v1[:, mc, :], ps[:], b1v[:, mc : mc + 1], 0.0, AL.add, AL.max
                )
            # conv2: 256 -> 300
            MW2 = (128, 128, 44)
            v2 = sb.tile([128, 3, 512], F32, tag="v2")
            for mc in range(3):
                mw = MW2[mc]
                ps = mmp.tile([128, 512], F32, tag="psv")
                for kc in range(2):
                    nc.tensor.matmul(
                        out=ps[0:mw, :],
                        lhsT=w2v[:, kc * 300 + mc * 128 : kc * 300 + mc * 128 + mw],
                        rhs=v1[:, kc, :],
                        start=(kc == 0),
                        stop=(kc == 1),
                    )
                nc.vector.tensor_scalar(
                    v2[0:mw, mc, :], ps[0:mw, :], b2v[0:mw, mc : mc + 1], 0.0,
                    AL.add, AL.max,
                )
            # conv3: 300 -> 300 (no relu)
            v3 = sb.tile([128, 3, 512], F32, tag="v3")
            for mc in range(3):
                mw = MW2[mc]
                ps = mmp.tile([128, 512], F32, tag="psv")
                for kc in range(3):
                    kw = MW2[kc]
                    nc.tensor.matmul(
                        out=ps[0:mw, :],
                        lhsT=w3v[0:kw, kc * 300 + mc * 128 : kc * 300 + mc * 128 + mw],
                        rhs=v2[0:kw, kc, :],
                        start=(kc == 0),
                        stop=(kc == 2),
                    )
                nc.vector.tensor_scalar(
                    v3[0:mw, mc, :], ps[0:mw, :], b3v[0:mw, mc : mc + 1], None, AL.add
                )

            # transpose scores to [pt, view] and emit
            vsT = sb.tile([128, 4, V], F32, tag="vsT")
            for f in range(4):
                for mc in range(3):
                    mw = MW2[mc]
                    tp = mmp.tile([128, 128], F32, tag="psv")
                    nc.tensor.transpose(
                        out=tp[0:128, 0:mw],
                        in_=v3[0:mw, mc, f * 128 : (f + 1) * 128],
                        identity=idsb[0:mw, 0:mw],
                    )
                    nc.vector.tensor_copy(
                        vsT[:, f, mc * 128 : mc * 128 + mw], tp[0:128, 0:mw]
                    )
            nc.sync.dma_start(
                vs_o.rearrange("(p f) v -> p f v", f=4), vsT[:]
            )

            # per-point view argmax (first-match, exact)
            tvi = sb.tile([128, 4], U32, tag="tvi")
            tvs = sb.tile([128, 4], F32, tag="tvs")
            mx8 = sb.tile([128, 8], F32, tag="mx8")
            mi8 = sb.tile([128, 8], U32, tag="mi8")
            for f in range(4):
                nc.vector.max(mx8[:], vsT[:, f, :])
                nc.vector.max_index(mi8[:], mx8[:], vsT[:, f, :])
                nc.vector.tensor_copy(tvs[:, f : f + 1], mx8[:, 0:1])
                nc.vector.tensor_copy(tvi[:, f : f + 1], mi8[:, 0:1])
            nc.sync.dma_start(tvi_o, tvi[:])
            nc.sync.dma_start(tvs_o, tvs[:])

            # vp_xyz gather + rotation matrices
            vx = sb.tile([128, 4, 3], F32, tag="vx")
            for f in range(4):
                nc.gpsimd.indirect_dma_start(
                    out=vx[:, f, :],
                    out_offset=None,
                    in_=tmpl_d,
                    in_offset=bass.IndirectOffsetOnAxis(ap=tvi[:, f : f + 1], axis=0),
                )
            nc.sync.dma_start(vpx_o, vx[:])

            t_t = sb.tile([128, 4, 3], F32, tag="t_t")
            sq3 = sb.tile([128, 4, 3], F32, tag="sq3")
            n2 = sb.tile([128, 4], F32, tag="n2")
            ny2 = sb.tile([128, 4], F32, tag="ny2")
            rn = sb.tile([128, 4], F32, tag="rn")
            rny = sb.tile([128, 4], F32, tag="rny")
            nrm = sb.tile([128, 4], F32, tag="nrm")
            nyrm = sb.tile([128, 4], F32, tag="nyrm")
            ax = sb.tile([128, 4, 3], F32, tag="ax")
            ay = sb.tile([128, 4, 3], F32, tag="ay")
            rot = sb.tile([128, 4, 9], F32, tag="rot")
            cm1 = sb.tile([128, 4], F32, tag="cm1")
            cm2 = sb.tile([128, 4], F32, tag="cm2")

            nc.vector.tensor_scalar(t_t[:], vx[:], -1.0, None, AL.mult)
            nc.vector.tensor_tensor(out=sq3[:], in0=t_t[:], in1=t_t[:], op=AL.mult)
            nc.vector.tensor_reduce(
                out=n2[:], in_=sq3[:], axis=mybir.AxisListType.X, op=AL.add
            )
            nc.vector.tensor_tensor(
                out=ny2[:],
                in0=sq3[:, :, 1].rearrange("p f -> p f"),
                in1=sq3[:, :, 0].rearrange("p f -> p f"),
                op=AL.add,
            )
            nc.scalar.sqrt(nrm[:], n2[:])
            nc.scalar.sqrt(nyrm[:], ny2[:])
            nc.vector.reciprocal(rn[:], nrm[:])
            nc.vector.reciprocal(rny[:], nyrm[:])
            # ax = t * (1/|t|)
            nc.vector.tensor_tensor(
                out=ax[:],
                in0=t_t[:],
                in1=rn[:].rearrange("p (f o) -> p f o", o=1).to_broadcast((128, 4, 3)),
                op=AL.mult,
            )
            # ay_pre = (-ty, tx, 0), then * (1/|ay_pre|)
            nc.vector.tensor_scalar(ay[:, :, 0:1], t_t[:, :, 1:2], -1.0, None, AL.mult)
            nc.vector.tensor_copy(ay[:, :, 1:2], t_t[:, :, 0:1])
            nc.vector.memset(ay[:, :, 2:3], 0.0)
            nc.vector.tensor_tensor(
                out=ay[:],
                in0=ay[:],
                in1=rny[:].rearrange("p (f o) -> p f o", o=1).to_broadcast((128, 4, 3)),
                op=AL.mult,
            )
            # rot columns: [:, :, 3*r + c] = axis_c[r]
            for r in range(3):
                nc.vector.tensor_copy(rot[:, :, 3 * r : 3 * r + 1], ax[:, :, r : r + 1])
                nc.vector.tensor_copy(
                    rot[:, :, 3 * r + 1 : 3 * r + 2], ay[:, :, r : r + 1]
                )
            # az = cross(ax, ay) into rot[:, :, 3r+2]
            for r, (i1, i2) in enumerate(((1, 2), (2, 0), (0, 1))):
                nc.vector.tensor_tensor(
                    out=cm1[:], in0=ax[:, :, i1 : i1 + 1], in1=ay[:, :, i2 : i2 + 1],
                    op=AL.mult,
                )
                nc.vector.tensor_tensor(
                    out=cm2[:], in0=ax[:, :, i2 : i2 + 1], in1=ay[:, :, i1 : i1 + 1],
                    op=AL.mult,
                )
                nc.vector.tensor_tensor(
                    out=rot[:, :, 3 * r + 2 : 3 * r + 3],
                    in0=cm1[:].rearrange("p (f o) -> p f o", o=1),
                    in1=cm2[:].rearrange("p (f o) -> p f o", o=1),
                    op=AL.subtract,
                )
            nc.sync.dma_start(vpr_o, rot[:])

            # graspable_features output [256, 512]
            nc.sync.dma_start(
                feat_o.rearrange("(cc p) n -> p cc n", p=128), fz[:]
            )
    nc.finalize()
    return nc


def _get_prog(key, builder):
    if key not in _prog_cache:
        _prog_cache[key] = builder()
    return _prog_cache[key]


def kernel(
    seed_xyz, seed_features, gh_w1, gh_b1, gh_g1, gh_be1, gh_m1, gh_v1, gh_w2, gh_b2,
    w1, b1, g1, be1, m1, v1, w2, b2, g2, be2, m2, v2, w3, b3,
):
    trace = bool(int(os.environ.get("KERNEL_TRACE", "0")))
    seed_xyz = np.asarray(seed_xyz, np.float32)
    seed_features = np.asarray(seed_features, np.float32)

    # ---- NEFF-A ----
    w1f, b1f = _fold_bn(np.asarray(gh_w1), np.asarray(gh_b1), np.asarray(gh_g1),
                        np.asarray(gh_be1), np.asarray(gh_m1), np.asarray(gh_v1))
    w1c = _chunk_lhsT(np.ascontiguousarray(w1f.T), 2, 256)
    b1c = _chunk_bias(b1f, 2)
    w2c = _chunk_lhsT(np.ascontiguousarray(np.asarray(gh_w2).T), 2, 3)
    b2c = np.ascontiguousarray(np.asarray(gh_b2).reshape(3, 1).astype(np.float32))

    nc_a = _get_prog("A", build_neff_a)
    in_maps_a = []
    for c in range(8):
        b_, h_ = c // 2, c % 2
        in_maps_a.append({
            "fsh": np.ascontiguousarray(
                seed_features[b_, :, h_ * NH : (h_ + 1) * NH]
            ),
            "w1c": w1c, "b1c": b1c, "w2c": w2c, "b2c": b2c,
        })
    res_a = run_bass_kernel_spmd(nc_a, in_maps_a, core_ids=list(range(8)),
                                 trace=trace)
    LAST_TIMES["A"] = res_a.exec_time_ns
    g = np.stack([
        np.concatenate([res_a.results[2 * b_]["g_o"], res_a.results[2 * b_ + 1]["g_o"]],
                       axis=1)
        for b_ in range(B)
    ])  # [B, 3, N]

    graspness_score = np.ascontiguousarray(g[:, 2])
    objectness_mask = g[:, 1] > g[:, 0]
    gmask = (g[:, 2] > np.float32(0.1)) & objectness_mask

    midxs, dist0s, Ms = [], [], []
    for b_ in range(B):
        mi = np.nonzero(gmask[b_])[0].astype(np.uint32)
        M = len(mi)
        assert 0 < M <= CCAP, f"masked count {M} outside capacity {CCAP}"
        pad = np.zeros(CCAP, np.uint32)
        pad[:M] = mi
        d0 = np.full(CCAP, -1.0, np.float32)
        d0[:M] = 1.0e10
        midxs.append(pad)
        dist0s.append(d0)
        Ms.append(M)
    T = max(2, max(Ms))

    # ---- NEFF-B ----
    wv1, bv1 = _fold_bn(np.asarray(w1), np.asarray(b1), np.asarray(g1),
                        np.asarray(be1), np.asarray(m1), np.asarray(v1))
    wv2, bv2 = _fold_bn(np.asarray(w2), np.asarray(b2), np.asarray(g2),
                        np.asarray(be2), np.asarray(m2), np.asarray(v2))
    w1vc = _chunk_lhsT(np.ascontiguousarray(wv1.T), 2, 256)
    b1vc = _chunk_bias(bv1, 2)
    w2vc = _chunk_lhsT(np.ascontiguousarray(wv2.T), 2, 300)
    b2vc = _chunk_bias(bv2, 3)
    w3vc = _chunk_lhsT(np.ascontiguousarray(np.asarray(w3).T.astype(np.float32)), 3, 300)
    b3vc = _chunk_bias(np.asarray(b3).astype(np.float32), 3)
    tmpl = _templates()
    ident = np.eye(128, dtype=np.float32)
    iota8 = (np.arange(P, dtype=np.float32) * FC).reshape(P, 1)

    nc_b = _get_prog(("B", T), lambda: build_neff_b(T))
    in_maps_b = []
    for c in range(8):
        b_, h_ = c // 2, c % 2
        in_maps_b.append({
            "xyz": np.ascontiguousarray(seed_xyz[b_]),
            "fT": np.ascontiguousarray(seed_features[b_].T),
            "g2f": np.ascontiguousarray(graspness_score[b_].reshape(N, 1)),
            "tmpl": tmpl,
            "midxP": midxs[b_].reshape(P, FC),
            "midxR": midxs[b_].reshape(1, CCAP),
            "midxC": midxs[b_].reshape(CCAP, 1),
            "dist0": dist0s[b_].reshape(P, FC),
            "hoff": np.array([[h_ * 512]], np.uint32),
            "ident": ident, "iota8": iota8,
            "w1v": w1vc, "b1v": b1vc, "w2v": w2vc, "b2v": b2vc,
            "w3v": w3vc, "b3v": b3vc,
        })
    res_b = run_bass_kernel_spmd(nc_b, in_maps_b, core_ids=list(range(8)),
                                 trace=trace)
    LAST_TIMES["B"] = res_b.exec_time_ns

    # ---- assemble outputs ----
    def halves(name):
        return [(res_b.results[2 * b_][name], res_b.results[2 * b_ + 1][name])
                for b_ in range(B)]

    inds = np.stack([res_b.results[2 * b_]["inds_o"][0].astype(np.int32)
                     for b_ in range(B)])
    graspable_xyz = np.stack([
        np.concatenate([a.reshape(512, 3), bb.reshape(512, 3)])
        for a, bb in halves("gxyz_o")
    ])
    def _unblk(x):  # device col f*128+q -> point col q*4+f
        return np.ascontiguousarray(
            x.reshape(CH, 4, 128).transpose(0, 2, 1).reshape(CH, 512)
        )
    graspable_features = np.stack([
        np.concatenate([_unblk(a), _unblk(bb)], axis=1)
        for a, bb in halves("feat_o")
    ])
    fp2_graspness = np.stack([
        np.concatenate([a.reshape(512), bb.reshape(512)]) for a, bb in halves("fp2_o")
    ])
    view_score = np.stack([
        np.concatenate([a, bb]) for a, bb in halves("vs_o")
    ])
    top_view_inds = np.stack([
        np.concatenate([a.reshape(512), bb.reshape(512)]).astype(np.int32)
        for a, bb in halves("tvi_o")
    ])
    top_view_scores = np.stack([
        np.concatenate([a.reshape(512), bb.reshape(512)]) for a, bb in halves("tvs_o")
    ])
    vp_xyz = np.stack([
        np.concatenate([a.reshape(512, 3), bb.reshape(512, 3)])
        for a, bb in halves("vpx_o")
    ])
    vp_rot = np.stack([
        np.concatenate([a.reshape(512, 3, 3), bb.reshape(512, 3, 3)])
        for a, bb in halves("vpr_o")
    ])

    return (graspness_score, objectness_mask, graspable_xyz, inds,
            graspable_features, fp2_graspness, view_score, top_view_inds,
            top_view_scores, vp_xyz, vp_rot)
